# revision 6
# baseline (speedup 1.0000x reference)
"""Trainium2 Bass kernel for CommonCrossAttentionWeights bidirectional cross-attention.

Sharding: 8 cores x 2 heads each (head-pair parallel). Each core computes
Q/V projections for its 128-dim head slice, the shared score matrix in both
orientations (bf16, exp without max-subtraction -- scores are provably small),
both softmax directions (mask folded multiplicatively into av), then an
AllToAll redistributes head-sharded attention outputs to row-sharded layout
for the output projections + bias + elementwise dot.
"""
import sys
sys.path.insert(0, "/opt/trn_rl_repo")
import numpy as np
import ml_dtypes

import concourse.bass as bass
import concourse.tile as tile
from concourse import bacc, mybir
from concourse.bass_utils import run_bass_kernel_spmd

BF16 = mybir.dt.bfloat16
F32 = mybir.dt.float32
EXP = mybir.ActivationFunctionType.Exp
LN = mybir.ActivationFunctionType.Ln
ADD = mybir.AluOpType.add
MULT = mybir.AluOpType.mult

B, NA, NV, D, H = 2, 512, 4096, 1024, 16
HD = D // H                      # 64
SCALE = HD ** (-0.5)             # 0.125
NCORES = 8
BN = B * NA                      # 1024 amr rows
BM = B * NV                      # 8192 vis rows
KT = D // 128                    # 8 contraction tiles
AMR_PER_CORE = BN // NCORES      # 128
VIS_PER_CORE = BM // NCORES      # 1024
SHARD = AMR_PER_CORE + VIS_PER_CORE  # 1152 cols per a2a shard

_COMPILED = None
LAST_RESULT = None
SPMD_WALL_S = None


def _build():
    nc = bacc.Bacc("TRN2", target_bir_lowering=False, debug=False,
                   enable_asserts=True, num_devices=NCORES)

    def din(name, shape, dt=BF16):
        return nc.dram_tensor(name, shape, dt, kind="ExternalInput").ap()

    afT = din("afT", [D, BN])
    vfT = din("vfT", [D, BM])
    wq = din("wq", [D, 128])
    wqv = din("wqv", [D, 128])
    wk = din("wk", [D, 128])
    wkv = din("wkv", [D, 128])
    woa = din("woa", [D, D])
    wov = din("wov", [D, D])
    maskf = din("maskf", [128, 8], F32)
    ba = din("ba", [128, 8], F32)
    bv = din("bv", [128, 8], F32)
    afT32 = din("afT32", [D, AMR_PER_CORE], F32)
    vfT32 = din("vfT32", [D, VIS_PER_CORE], F32)

    o_amrT = nc.dram_tensor("o_amrT", [D, AMR_PER_CORE], F32, kind="ExternalOutput").ap()
    o_visT = nc.dram_tensor("o_visT", [D, VIS_PER_CORE], F32, kind="ExternalOutput").ap()

    r8 = lambda ap: ap.rearrange("(t p) c -> p t c", p=128)

    with tile.TileContext(nc) as tc:
        with tc.tile_pool(name="persist", bufs=1) as pers, \
             tc.tile_pool(name="psum", bufs=1, space="PSUM") as pp, \
             tc.tile_pool(name="dram", bufs=1, space="DRAM") as dram:

            # ---- persistent loads ----
            afT_sb = pers.tile([128, KT, BN], BF16)
            nc.sync.dma_start(afT_sb[:], r8(afT))
            wq_sb = pers.tile([128, KT, 128], BF16)
            nc.sync.dma_start(wq_sb[:], r8(wq))
            wqv_sb = pers.tile([128, KT, 128], BF16)
            nc.sync.dma_start(wqv_sb[:], r8(wqv))
            wk_sb = pers.tile([128, KT, 128], BF16)
            nc.sync.dma_start(wk_sb[:], r8(wk))
            wkv_sb = pers.tile([128, KT, 128], BF16)
            nc.sync.dma_start(wkv_sb[:], r8(wkv))
            maskf_sb = pers.tile([128, 8], F32)
            nc.sync.dma_start(maskf_sb[:], maskf[:])
            ba_sb = pers.tile([128, 8], F32)
            nc.sync.dma_start(ba_sb[:], ba[:])
            bv_sb = pers.tile([128, 8], F32)
            nc.sync.dma_start(bv_sb[:], bv[:])

            bounce = dram.tile([NCORES * 128, SHARD], BF16)
            gathered = dram.tile([NCORES * 128, SHARD], BF16)

            with tc.tile_pool(name="ph12", bufs=1) as p12:
                # ---- phase 1: projections ----
                aqT_sb = p12.tile([128, BN], BF16)
                vqT_sb = p12.tile([128, 16, 512], BF16)
                vv_sb = p12.tile([128, 64, 130], BF16)
                av_sb = p12.tile([128, 8, 130], BF16)

                for half in range(2):
                    ps = pp.tile([128, 512], F32, tag="p1", bufs=2, name="ps_aq")
                    for kt in range(KT):
                        nc.tensor.matmul(ps[:], wq_sb[:, kt, :],
                                         afT_sb[:, kt, 512 * half:512 * half + 512],
                                         start=kt == 0, stop=kt == KT - 1)
                    nc.vector.tensor_copy(aqT_sb[:, 512 * half:512 * half + 512], ps[:])

                for nt in range(8):
                    ps = pp.tile([128, 128], F32, tag="p1", bufs=2, name="ps_av")
                    for kt in range(KT):
                        nc.tensor.matmul(ps[:], afT_sb[:, kt, 128 * nt:128 * nt + 128],
                                         wqv_sb[:, kt, :], start=kt == 0, stop=kt == KT - 1)
                    for h in range(2):
                        nc.vector.tensor_scalar(out=av_sb[:, nt, 65 * h:65 * h + 64],
                                                in0=ps[:, 64 * h:64 * h + 64],
                                                scalar1=maskf_sb[:, nt:nt + 1],
                                                scalar2=None, op0=MULT)
                        nc.vector.tensor_copy(av_sb[:, nt, 65 * h + 64:65 * h + 65],
                                              maskf_sb[:, nt:nt + 1])

                for mb in range(16):
                    blk = p12.tile([128, KT, 512], BF16, tag="vfT", bufs=2, name="vf_blk")
                    nc.sync.dma_start(blk[:], r8(vfT)[:, :, 512 * mb:512 * mb + 512])
                    ps = pp.tile([128, 512], F32, tag="p1", bufs=2, name="ps_vq")
                    for kt in range(KT):
                        nc.tensor.matmul(ps[:], wk_sb[:, kt, :], blk[:, kt, :],
                                         start=kt == 0, stop=kt == KT - 1)
                    nc.vector.tensor_copy(vqT_sb[:, mb, :], ps[:])
                    for r in range(4):
                        ps2 = pp.tile([128, 128], F32, tag="p1", bufs=2, name="ps_vv")
                        for kt in range(KT):
                            nc.tensor.matmul(ps2[:], blk[:, kt, 128 * r:128 * r + 128],
                                             wkv_sb[:, kt, :], start=kt == 0, stop=kt == KT - 1)
                        mt = 4 * mb + r
                        nc.vector.tensor_copy(vv_sb[:, mt, 0:64], ps2[:, 0:64])
                        nc.vector.tensor_copy(vv_sb[:, mt, 65:129], ps2[:, 64:128])
                nc.vector.memset(vv_sb[:, :, 64:65], 1.0)
                nc.vector.memset(vv_sb[:, :, 129:130], 1.0)

                # ---- phase 2: attention per (b, h) ----
                amr2n = [p12.tile([64, BN], BF16, name=f"amr2n_h{h}") for h in range(2)]
                vis2n = [p12.tile([64, 16, 512], BF16, name=f"vis2n_h{h}") for h in range(2)]

                for b in range(B):
                    for h in range(2):
                        hs = 64 * h
                        vis2u = p12.tile([64, 8, 512], F32, tag="vis2u", bufs=1, name="vis2u")
                        stage = p12.tile([65, 9, 512], F32, tag="stage", bufs=1, name="stage")
                        rows9 = p12.tile([9, 512], F32, tag="rows9", bufs=2, name="rows9")

                        # pass A: E = exp(scores[n,m]) feeding dir-2 (vis attends amr)
                        for mc in range(4):
                            e_tiles = []
                            for nt in range(4):
                                sc = pp.tile([128, 1024], F32, tag="sc", bufs=2, name="sc_ps")
                                for j in range(2):
                                    mbl = 8 * b + 2 * mc + j
                                    nc.tensor.matmul(
                                        sc[:, 512 * j:512 * j + 512],
                                        aqT_sb[hs:hs + 64, 512 * b + 128 * nt:512 * b + 128 * nt + 128],
                                        vqT_sb[hs:hs + 64, mbl, :],
                                        start=True, stop=True)
                                et = p12.tile([128, 1024], BF16, tag="E", bufs=6, name="E_t")
                                nc.scalar.activation(et[:], sc[:], EXP, scale=SCALE)
                                e_tiles.append(et)
                            for j in range(2):
                                cc = 2 * mc + j
                                v2 = pp.tile([65, 512], F32, tag="v2", bufs=2, name="v2_ps")
                                for nt in range(4):
                                    nc.tensor.matmul(v2[:], av_sb[:, 4 * b + nt, 65 * h:65 * h + 65],
                                                     e_tiles[nt][:, 512 * j:512 * j + 512],
                                                     start=nt == 0, stop=nt == 3)
                                nc.vector.tensor_copy(stage[64:65, cc, :], v2[64:65, :])
                                nc.vector.tensor_copy(vis2u[:, cc, :], v2[0:64, :])

                        # pass B: E^T = exp(scores[m,n]) feeding dir-1 (amr attends vis)
                        a2 = pp.tile([65, 512], F32, tag="v2", bufs=2, name="a2_ps")
                        for mt in range(32):
                            scT = pp.tile([128, 512], F32, tag="p1", bufs=2, name="scT_ps")
                            mbl = 8 * b + mt // 4
                            r = mt % 4
                            nc.tensor.matmul(scT[:],
                                             vqT_sb[hs:hs + 64, mbl, 128 * r:128 * r + 128],
                                             aqT_sb[hs:hs + 64, 512 * b:512 * b + 512],
                                             start=True, stop=True)
                            ett = p12.tile([128, 512], BF16, tag="ET", bufs=4, name="ET_t")
                            nc.scalar.activation(ett[:], scT[:], EXP, scale=SCALE)
                            nc.tensor.matmul(a2[:], vv_sb[:, 32 * b + mt, 65 * h:65 * h + 65],
                                             ett[:], start=mt == 0, stop=mt == 31)
                        nc.vector.tensor_copy(stage[64:65, 8, :], a2[64:65, :])

                        # denominators -> rows on distinct partitions -> 1/x = exp(-ln(x))
                        nc.sync.dma_start(rows9[0:9, :], stage[64:65, :, :])
                        rln = p12.tile([9, 512], F32, tag="rln", bufs=2, name="rln")
                        nc.scalar.activation(rln[:], rows9[0:9, :], LN)
                        rrec = p12.tile([9, 512], F32, tag="rrec", bufs=2, name="rrec")
                        nc.scalar.activation(rrec[:], rln[:], EXP, scale=-1.0)

                        p0r = p12.tile([1, 512], F32, tag="p0r", bufs=2, name="p0r_amr")
                        nc.sync.dma_start(p0r[:], rrec[8:9, :])
                        bc = p12.tile([64, 512], F32, tag="bc", bufs=4, name="bc_amr")
                        nc.gpsimd.partition_broadcast(bc[:], p0r[:])
                        nc.vector.tensor_mul(amr2n[h][0:64, 512 * b:512 * b + 512],
                                             a2[0:64, :], bc[:])
                        for cc in range(8):
                            p0r2 = p12.tile([1, 512], F32, tag="p0r", bufs=2, name="p0r_vis")
                            nc.sync.dma_start(p0r2[:], rrec[cc:cc + 1, :])
                            bc2 = p12.tile([64, 512], F32, tag="bc", bufs=4, name="bc_vis")
                            nc.gpsimd.partition_broadcast(bc2[:], p0r2[:])
                            nc.vector.tensor_mul(vis2n[h][0:64, 8 * b + cc, :],
                                                 vis2u[:, cc, :], bc2[:])

                # ---- phase 3: assemble a2a shards (dest-major) + AllToAll ----
                for j in range(NCORES):
                    for h in range(2):
                        ro = 128 * j + 64 * h
                        nc.sync.dma_start(bounce[ro:ro + 64, 0:128],
                                          amr2n[h][0:64, 128 * j:128 * j + 128])
                        nc.sync.dma_start(bounce[ro:ro + 64, 128:SHARD],
                                          vis2n[h][0:64, 2 * j:2 * j + 2, :])
                nc.gpsimd.collective_compute(
                    "AllToAll", mybir.AluOpType.bypass,
                    replica_groups=[list(range(NCORES))],
                    ins=[bounce.opt()], outs=[gathered.opt()])

            # ---- phase 4: row-sharded output projections + bias + dot ----
            with tc.tile_pool(name="ph4", bufs=1) as p4:
                woa_sb = p4.tile([128, KT, D], BF16)
                nc.sync.dma_start(woa_sb[:], r8(woa))
                wov_sb = p4.tile([128, KT, D], BF16)
                nc.sync.dma_start(wov_sb[:], r8(wov))
                amrg = p4.tile([128, KT, 128], BF16)
                nc.sync.dma_start(amrg[:], r8(gathered)[:, :, 0:128])
                visg = p4.tile([128, KT, 1024], BF16)
                nc.sync.dma_start(visg[:], r8(gathered)[:, :, 128:SHARD])
                afT32_sb = p4.tile([128, KT, 128], F32)
                nc.sync.dma_start(afT32_sb[:], r8(afT32))

                for dt in range(8):
                    psa = pp.tile([128, 128], F32, tag="p1", bufs=2, name="psa")
                    for kt in range(KT):
                        nc.tensor.matmul(psa[:], woa_sb[:, kt, 128 * dt:128 * dt + 128],
                                         amrg[:, kt, :], start=kt == 0, stop=kt == KT - 1)
                    st = p4.tile([128, 128], F32, tag="oa", bufs=3, name="oa_st")
                    nc.vector.tensor_scalar(out=st[:], in0=psa[:],
                                            scalar1=ba_sb[:, dt:dt + 1], scalar2=None, op0=ADD)
                    nc.vector.tensor_mul(st[:], st[:], afT32_sb[:, dt, :])
                    nc.sync.dma_start(r8(o_amrT)[:, dt, :], st[:])

                    vf = p4.tile([128, 1024], F32, tag="vf32", bufs=2, name="vf32")
                    nc.sync.dma_start(vf[:], r8(vfT32)[:, dt, :])
                    for half in range(2):
                        psv = pp.tile([128, 512], F32, tag="p1", bufs=2, name="psv")
                        for kt in range(KT):
                            nc.tensor.matmul(psv[:], wov_sb[:, kt, 128 * dt:128 * dt + 128],
                                             visg[:, kt, 512 * half:512 * half + 512],
                                             start=kt == 0, stop=kt == KT - 1)
                        st2 = p4.tile([128, 512], F32, tag="ov", bufs=3, name="ov_st")
                        nc.vector.tensor_scalar(out=st2[:], in0=psv[:],
                                                scalar1=bv_sb[:, dt:dt + 1], scalar2=None, op0=ADD)
                        nc.vector.tensor_mul(st2[:], st2[:], vf[:, 512 * half:512 * half + 512])
                        nc.sync.dma_start(r8(o_visT)[:, dt, 512 * half:512 * half + 512], st2[:])

    nc.compile()
    return nc


def _bf16(x):
    return np.ascontiguousarray(x).astype(ml_dtypes.bfloat16)


def kernel(amr_feats, amr_pad_mask, visual_feats,
           W_amr, W_amr_v, W_vis, W_vis_v,
           W_amr_out, b_amr_out, W_vis_out, b_vis_out,
           _trace=False):
    global _COMPILED, LAST_RESULT
    if _COMPILED is None:
        _COMPILED = _build()
    nc = _COMPILED

    amr_feats = np.asarray(amr_feats, np.float32)
    visual_feats = np.asarray(visual_feats, np.float32)
    amr_pad_mask = np.asarray(amr_pad_mask)
    W = {k: np.asarray(v, np.float32) for k, v in dict(
        W_amr=W_amr, W_amr_v=W_amr_v, W_vis=W_vis, W_vis_v=W_vis_v,
        W_amr_out=W_amr_out, W_vis_out=W_vis_out).items()}
    b_amr_out = np.asarray(b_amr_out, np.float32)
    b_vis_out = np.asarray(b_vis_out, np.float32)

    afT32 = np.ascontiguousarray(amr_feats.reshape(BN, D).T)      # [D, BN]
    vfT32 = np.ascontiguousarray(visual_feats.reshape(BM, D).T)   # [D, BM]
    afT = _bf16(afT32)
    vfT = _bf16(vfT32)
    woa = _bf16(W["W_amr_out"])
    wov = _bf16(W["W_vis_out"])
    mask0 = (~amr_pad_mask.reshape(BN)).astype(np.float32)
    maskf = np.ascontiguousarray(mask0.reshape(8, 128).T)
    ba = np.ascontiguousarray(b_amr_out.reshape(8, 128).T)
    bv = np.ascontiguousarray(b_vis_out.reshape(8, 128).T)

    in_maps = []
    for j in range(NCORES):
        cs = slice(128 * j, 128 * (j + 1))
        in_maps.append({
            "afT": afT, "vfT": vfT,
            "wq": _bf16(W["W_amr"][:, cs]), "wqv": _bf16(W["W_amr_v"][:, cs]),
            "wk": _bf16(W["W_vis"][:, cs]), "wkv": _bf16(W["W_vis_v"][:, cs]),
            "woa": woa, "wov": wov,
            "maskf": maskf, "ba": ba, "bv": bv,
            "afT32": np.ascontiguousarray(afT32[:, 128 * j:128 * (j + 1)]),
            "vfT32": np.ascontiguousarray(vfT32[:, 1024 * j:1024 * (j + 1)]),
        })

    import time as _time
    _t0 = _time.time()
    res = run_bass_kernel_spmd(nc, in_maps, core_ids=list(range(NCORES)), trace=_trace)
    global SPMD_WALL_S
    SPMD_WALL_S = _time.time() - _t0
    LAST_RESULT = res

    amrT = np.concatenate([res.results[j]["o_amrT"] for j in range(NCORES)], axis=1)
    visT = np.concatenate([res.results[j]["o_visT"] for j in range(NCORES)], axis=1)
    out_amr = np.ascontiguousarray(amrT.T).reshape(B, NA, D)
    out_vis = np.ascontiguousarray(visT.T).reshape(B, NV, D)
    return out_amr, out_vis


# revision 16
# speedup vs baseline: 1.2880x; 1.2880x over previous
"""Trainium2 Bass kernel for CommonCrossAttentionWeights bidirectional cross-attention.

Sharding: 8 cores x 2 heads each (head-pair parallel). Each core computes
Q/V projections for its 128-dim head slice, the shared score matrix in both
orientations (bf16, exp without max-subtraction -- scores are provably small),
both softmax directions (mask folded multiplicatively into av), then an
AllToAll redistributes head-sharded attention outputs to row-sharded layout
for the output projections + bias + elementwise dot.
"""
import sys
sys.path.insert(0, "/opt/trn_rl_repo")
import numpy as np
import ml_dtypes

import concourse.bass as bass
import concourse.tile as tile
from concourse import bacc, mybir
from concourse.bass_utils import run_bass_kernel_spmd

BF16 = mybir.dt.bfloat16
F32 = mybir.dt.float32
EXP = mybir.ActivationFunctionType.Exp
LN = mybir.ActivationFunctionType.Ln
ADD = mybir.AluOpType.add
MULT = mybir.AluOpType.mult

B, NA, NV, D, H = 2, 512, 4096, 1024, 16
HD = D // H                      # 64
SCALE = HD ** (-0.5)             # 0.125
NCORES = 8
BN = B * NA                      # 1024 amr rows
BM = B * NV                      # 8192 vis rows
KT = D // 128                    # 8 contraction tiles
AMR_PER_CORE = BN // NCORES      # 128
VIS_PER_CORE = BM // NCORES      # 1024
SHARD = AMR_PER_CORE + VIS_PER_CORE  # 1152 cols per a2a shard

_COMPILED = None
LAST_RESULT = None
SPMD_WALL_S = None


def _build(fake_cc=False):
    nc = bacc.Bacc("TRN2", target_bir_lowering=False, debug=False,
                   enable_asserts=True, num_devices=NCORES)

    def din(name, shape, dt=BF16):
        return nc.dram_tensor(name, shape, dt, kind="ExternalInput").ap()

    afT = din("afT", [D, BN])
    vfT = din("vfT", [D, BM])
    wq = din("wq", [D, 128])
    wqv = din("wqv", [D, 128])
    wk = din("wk", [D, 128])
    wkv = din("wkv", [D, 128])
    woa = din("woa", [D, D])
    wov = din("wov", [D, D])
    maskf = din("maskf", [128, 8], F32)
    ba = din("ba", [128, 8], F32)
    bv = din("bv", [128, 8], F32)
    afT32 = din("afT32", [D, AMR_PER_CORE], F32)
    vfT32 = din("vfT32", [D, VIS_PER_CORE], F32)

    o_amrT = nc.dram_tensor("o_amrT", [D, AMR_PER_CORE], F32, kind="ExternalOutput").ap()
    o_visT = nc.dram_tensor("o_visT", [D, VIS_PER_CORE], F32, kind="ExternalOutput").ap()

    r8 = lambda ap: ap.rearrange("(t p) c -> p t c", p=128)

    with tile.TileContext(nc) as tc:
        with tc.tile_pool(name="persist", bufs=1) as pers, \
             tc.tile_pool(name="psum", bufs=1, space="PSUM") as pp, \
             tc.tile_pool(name="dram", bufs=1, space="DRAM") as dram:

            # ---- persistent loads ----
            wq_sb = pers.tile([128, KT, 128], BF16)
            nc.sync.dma_start(wq_sb[:], r8(wq))
            wk_sb = pers.tile([128, KT, 128], BF16)
            nc.gpsimd.dma_start(wk_sb[:], r8(wk))
            wkv_sb = pers.tile([128, KT, 128], BF16)
            nc.gpsimd.dma_start(wkv_sb[:], r8(wkv))
            wqv_sb = pers.tile([128, KT, 128], BF16)
            nc.sync.dma_start(wqv_sb[:], r8(wqv))
            afT_sb = pers.tile([128, KT, BN], BF16)
            for kt in range(KT):
                eng = nc.gpsimd if kt % 2 == 0 else nc.sync
                eng.dma_start(afT_sb[:, kt, :], r8(afT)[:, kt, :])
            woa_sb = pers.tile([128, KT, D], BF16)
            wov_sb = pers.tile([128, KT, D], BF16)
            for kt in range(KT):
                eng = nc.sync if kt % 2 == 0 else nc.gpsimd
                eng.dma_start(woa_sb[:, kt, :], r8(woa)[:, kt, :])
                eng2 = nc.gpsimd if kt % 2 == 0 else nc.sync
                eng2.dma_start(wov_sb[:, kt, :], r8(wov)[:, kt, :])
            maskf_sb = pers.tile([128, 8], F32)
            nc.sync.dma_start(maskf_sb[:], maskf[:])
            ba_sb = pers.tile([128, 8], F32)
            nc.sync.dma_start(ba_sb[:], ba[:])
            bv_sb = pers.tile([128, 8], F32)
            nc.sync.dma_start(bv_sb[:], bv[:])

            bounce = dram.tile([NCORES * 128, SHARD], BF16)
            gathered = dram.tile([NCORES * 128, SHARD], BF16)

            with tc.tile_pool(name="ph12", bufs=1) as p12:
                # ---- phase 1: projections ----
                aqT_sb = p12.tile([128, BN], BF16)
                vqT_sb = p12.tile([128, 16, 512], BF16)
                vv_sb = p12.tile([128, 64, 130], BF16)
                av_sb = p12.tile([128, 8, 130], BF16)

                for half in range(2):
                    ps = pp.tile([128, 512], F32, tag="p1", bufs=2, name="ps_aq")
                    for kt in range(KT):
                        nc.tensor.matmul(ps[:], wq_sb[:, kt, :],
                                         afT_sb[:, kt, 512 * half:512 * half + 512],
                                         start=kt == 0, stop=kt == KT - 1)
                    nc.vector.tensor_copy(aqT_sb[:, 512 * half:512 * half + 512], ps[:])

                for nt in range(8):
                    ps = pp.tile([128, 128], F32, tag="p1", bufs=2, name="ps_av")
                    for kt in range(KT):
                        nc.tensor.matmul(ps[:], afT_sb[:, kt, 128 * nt:128 * nt + 128],
                                         wqv_sb[:, kt, :], start=kt == 0, stop=kt == KT - 1)
                    for h in range(2):
                        nc.vector.tensor_scalar(out=av_sb[:, nt, 65 * h:65 * h + 64],
                                                in0=ps[:, 64 * h:64 * h + 64],
                                                scalar1=maskf_sb[:, nt:nt + 1],
                                                scalar2=None, op0=MULT)
                        nc.vector.tensor_copy(av_sb[:, nt, 65 * h + 64:65 * h + 65],
                                              maskf_sb[:, nt:nt + 1])

                for mb in range(16):
                    blk = p12.tile([128, KT, 512], BF16, tag="vfT", bufs=3, name="vf_blk")
                    dma_eng = nc.sync if mb % 2 == 0 else nc.gpsimd
                    dma_eng.dma_start(blk[:], r8(vfT)[:, :, 512 * mb:512 * mb + 512])
                    ps = pp.tile([128, 512], F32, tag="p1", bufs=2, name="ps_vq")
                    for kt in range(KT):
                        nc.tensor.matmul(ps[:], wk_sb[:, kt, :], blk[:, kt, :],
                                         start=kt == 0, stop=kt == KT - 1)
                    nc.vector.tensor_copy(vqT_sb[:, mb, :], ps[:])
                    for r in range(4):
                        ps2 = pp.tile([128, 128], F32, tag="p1", bufs=2, name="ps_vv")
                        for kt in range(KT):
                            nc.tensor.matmul(ps2[:], blk[:, kt, 128 * r:128 * r + 128],
                                             wkv_sb[:, kt, :], start=kt == 0, stop=kt == KT - 1)
                        mt = 4 * mb + r
                        nc.vector.tensor_copy(vv_sb[:, mt, 0:64], ps2[:, 0:64])
                        nc.vector.tensor_copy(vv_sb[:, mt, 65:129], ps2[:, 64:128])
                nc.vector.memset(vv_sb[:, :, 64:65], 1.0)
                nc.vector.memset(vv_sb[:, :, 129:130], 1.0)

                # ---- phase 2: attention per (b, h) ----
                amr2n = [p12.tile([64, BN], BF16, name=f"amr2n_h{h}") for h in range(2)]

                for b in range(B):
                    for h in range(2):
                        hs = 64 * h
                        vis2u = p12.tile([64, 8, 512], F32, tag="vis2u", bufs=2, name="vis2u")
                        stage = p12.tile([65, 9, 512], F32, tag="stage", bufs=1, name="stage")
                        rows9 = p12.tile([9, 512], F32, tag="rows9", bufs=2, name="rows9")

                        # pass A: E = exp(scores[n,m]) feeding dir-2 (vis attends amr)
                        for mc in range(4):
                            e_tiles = []
                            for nt in range(4):
                                sc = pp.tile([128, 1024], F32, tag="sc", bufs=2, name="sc_ps")
                                for j in range(2):
                                    mbl = 8 * b + 2 * mc + j
                                    nc.tensor.matmul(
                                        sc[:, 512 * j:512 * j + 512],
                                        aqT_sb[hs:hs + 64, 512 * b + 128 * nt:512 * b + 128 * nt + 128],
                                        vqT_sb[hs:hs + 64, mbl, :],
                                        start=True, stop=True)
                                et = p12.tile([128, 1024], BF16, tag="E", bufs=5, name="E_t")
                                nc.scalar.activation(et[:], sc[:], EXP, scale=SCALE)
                                e_tiles.append(et)
                            for j in range(2):
                                cc = 2 * mc + j
                                v2 = pp.tile([65, 512], F32, tag="v2", bufs=2, name="v2_ps")
                                for nt in range(4):
                                    nc.tensor.matmul(v2[:], av_sb[:, 4 * b + nt, 65 * h:65 * h + 65],
                                                     e_tiles[nt][:, 512 * j:512 * j + 512],
                                                     start=nt == 0, stop=nt == 3)
                                nc.vector.tensor_copy(stage[64:65, cc, :], v2[64:65, :])
                                nc.vector.tensor_copy(vis2u[:, cc, :], v2[0:64, :])

                        # pass B: E^T = exp(scores[m,n]) feeding dir-1 (amr attends vis)
                        a2 = pp.tile([65, 512], F32, tag="v2", bufs=2, name="a2_ps")
                        for mp in range(16):
                            scT = pp.tile([128, 1024], F32, tag="sc", bufs=2, name="scT_ps")
                            for q in range(2):
                                mt = 2 * mp + q
                                mbl = 8 * b + mt // 4
                                r = mt % 4
                                nc.tensor.matmul(scT[:, 512 * q:512 * q + 512],
                                                 vqT_sb[hs:hs + 64, mbl, 128 * r:128 * r + 128],
                                                 aqT_sb[hs:hs + 64, 512 * b:512 * b + 512],
                                                 start=True, stop=True)
                            ett = p12.tile([128, 1024], BF16, tag="ET", bufs=2, name="ET_t")
                            nc.scalar.activation(ett[:], scT[:], EXP, scale=SCALE)
                            for q in range(2):
                                mt = 2 * mp + q
                                nc.tensor.matmul(a2[:], vv_sb[:, 32 * b + mt, 65 * h:65 * h + 65],
                                                 ett[:, 512 * q:512 * q + 512],
                                                 start=mt == 0, stop=mt == 31)
                        nc.vector.tensor_copy(stage[64:65, 8, :], a2[64:65, :])
                        a2u = p12.tile([64, 512], F32, tag="a2u", bufs=2, name="a2u")
                        nc.vector.tensor_copy(a2u[:], a2[0:64, :])

                        # denominators -> rows on distinct partitions -> fast reciprocal
                        nc.sync.dma_start(rows9[0:9, :], stage[64:65, :, :])
                        rrec = p12.tile([9, 512], F32, tag="rrec", bufs=2, name="rrec")
                        nc.vector.reciprocal_approx_fast(out=rrec[:], in_=rows9[0:9, :])

                        p0r = p12.tile([1, 512], F32, tag="p0r", bufs=1, name="p0r_amr")
                        nc.sync.dma_start(p0r[:], rrec[8:9, :])
                        bc = p12.tile([64, 512], F32, tag="bc", bufs=3, name="bc_amr")
                        nc.gpsimd.partition_broadcast(bc[:], p0r[:])
                        nc.vector.tensor_mul(amr2n[h][0:64, 512 * b:512 * b + 512],
                                             a2u[:], bc[:])
                        for cc in range(8):
                            p0r2 = p12.tile([1, 512], F32, tag="p0r", bufs=1, name="p0r_vis")
                            nc.sync.dma_start(p0r2[:], rrec[cc:cc + 1, :])
                            bc2 = p12.tile([64, 512], F32, tag="bc", bufs=3, name="bc_vis")
                            nc.gpsimd.partition_broadcast(bc2[:], p0r2[:])
                            v2n = p12.tile([64, 512], BF16, tag="v2n", bufs=3, name="v2n")
                            nc.vector.tensor_mul(v2n[:], vis2u[:, cc, :], bc2[:])
                            j = 4 * b + cc // 2
                            ro = 128 * j + 64 * h
                            co = 128 + 512 * (cc % 2)
                            nc.gpsimd.dma_start(bounce[ro:ro + 64, co:co + 512], v2n[:])
                        # drain this (b,h)'s amr slice of the a2a bounce buffer
                        for j in range(4 * b, 4 * b + 4):
                            ro = 128 * j + 64 * h
                            nc.sync.dma_start(bounce[ro:ro + 64, 0:128],
                                              amr2n[h][0:64, 128 * j:128 * j + 128])

                # ---- phase 3: AllToAll (bounce was drained inside the loop) ----
                if fake_cc:
                    nc.sync.dma_start(gathered[:], bounce[:])
                else:
                    nc.gpsimd.collective_compute(
                        "AllToAll", mybir.AluOpType.bypass,
                        replica_groups=[list(range(NCORES))],
                        ins=[bounce.opt()], outs=[gathered.opt()])

            # ---- phase 4: row-sharded output projections + bias + dot ----
            with tc.tile_pool(name="ph4", bufs=1) as p4:
                amrg = p4.tile([128, KT, 128], BF16)
                nc.sync.dma_start(amrg[:], r8(gathered)[:, :, 0:128])
                visg = p4.tile([128, KT, 1024], BF16)
                for kt in range(KT):
                    eng = nc.sync if kt % 2 == 0 else nc.gpsimd
                    eng.dma_start(visg[:, kt, :], r8(gathered)[:, kt, 128:SHARD])
                afT32_sb = p4.tile([128, KT, 128], F32)
                nc.gpsimd.dma_start(afT32_sb[:], r8(afT32))

                for dt in range(8):
                    psa = pp.tile([128, 128], F32, tag="p1", bufs=2, name="psa")
                    for kt in range(KT):
                        nc.tensor.matmul(psa[:], woa_sb[:, kt, 128 * dt:128 * dt + 128],
                                         amrg[:, kt, :], start=kt == 0, stop=kt == KT - 1)
                    st = p4.tile([128, 128], F32, tag="oa", bufs=3, name="oa_st")
                    nc.vector.tensor_scalar(out=st[:], in0=psa[:],
                                            scalar1=ba_sb[:, dt:dt + 1], scalar2=None, op0=ADD)
                    nc.vector.tensor_mul(st[:], st[:], afT32_sb[:, dt, :])
                    nc.sync.dma_start(r8(o_amrT)[:, dt, :], st[:])

                for dt in range(8):
                    vf = p4.tile([128, 1024], F32, tag="vf32", bufs=2, name="vf32")
                    nc.gpsimd.dma_start(vf[:], r8(vfT32)[:, dt, :])
                    for half in range(2):
                        psv = pp.tile([128, 512], F32, tag="p1", bufs=2, name="psv")
                        for kt in range(KT):
                            nc.tensor.matmul(psv[:], wov_sb[:, kt, 128 * dt:128 * dt + 128],
                                             visg[:, kt, 512 * half:512 * half + 512],
                                             start=kt == 0, stop=kt == KT - 1)
                        st2 = p4.tile([128, 512], F32, tag="ov", bufs=3, name="ov_st")
                        nc.vector.tensor_scalar(out=st2[:], in0=psv[:],
                                                scalar1=bv_sb[:, dt:dt + 1], scalar2=None, op0=ADD)
                        nc.vector.tensor_mul(st2[:], st2[:], vf[:, 512 * half:512 * half + 512])
                        nc.sync.dma_start(r8(o_visT)[:, dt, 512 * half:512 * half + 512], st2[:])

    nc.compile()
    return nc


def _bf16(x):
    return np.ascontiguousarray(x).astype(ml_dtypes.bfloat16)


def kernel(amr_feats, amr_pad_mask, visual_feats,
           W_amr, W_amr_v, W_vis, W_vis_v,
           W_amr_out, b_amr_out, W_vis_out, b_vis_out,
           _trace=False):
    global _COMPILED, LAST_RESULT
    if _COMPILED is None:
        _COMPILED = _build()
    nc = _COMPILED

    amr_feats = np.asarray(amr_feats, np.float32)
    visual_feats = np.asarray(visual_feats, np.float32)
    amr_pad_mask = np.asarray(amr_pad_mask)
    W = {k: np.asarray(v, np.float32) for k, v in dict(
        W_amr=W_amr, W_amr_v=W_amr_v, W_vis=W_vis, W_vis_v=W_vis_v,
        W_amr_out=W_amr_out, W_vis_out=W_vis_out).items()}
    b_amr_out = np.asarray(b_amr_out, np.float32)
    b_vis_out = np.asarray(b_vis_out, np.float32)

    afT32 = np.ascontiguousarray(amr_feats.reshape(BN, D).T)      # [D, BN]
    vfT32 = np.ascontiguousarray(visual_feats.reshape(BM, D).T)   # [D, BM]
    afT = _bf16(afT32)
    vfT = _bf16(vfT32)
    woa = _bf16(W["W_amr_out"])
    wov = _bf16(W["W_vis_out"])
    mask0 = (~amr_pad_mask.reshape(BN)).astype(np.float32)
    maskf = np.ascontiguousarray(mask0.reshape(8, 128).T)
    ba = np.ascontiguousarray(b_amr_out.reshape(8, 128).T)
    bv = np.ascontiguousarray(b_vis_out.reshape(8, 128).T)

    in_maps = []
    for j in range(NCORES):
        cs = slice(128 * j, 128 * (j + 1))
        in_maps.append({
            "afT": afT, "vfT": vfT,
            "wq": _bf16(W["W_amr"][:, cs]), "wqv": _bf16(W["W_amr_v"][:, cs]),
            "wk": _bf16(W["W_vis"][:, cs]), "wkv": _bf16(W["W_vis_v"][:, cs]),
            "woa": woa, "wov": wov,
            "maskf": maskf, "ba": ba, "bv": bv,
            "afT32": np.ascontiguousarray(afT32[:, 128 * j:128 * (j + 1)]),
            "vfT32": np.ascontiguousarray(vfT32[:, 1024 * j:1024 * (j + 1)]),
        })

    import time as _time
    _t0 = _time.time()
    res = run_bass_kernel_spmd(nc, in_maps, core_ids=list(range(NCORES)), trace=_trace)
    global SPMD_WALL_S
    SPMD_WALL_S = _time.time() - _t0
    LAST_RESULT = res

    amrT = np.concatenate([res.results[j]["o_amrT"] for j in range(NCORES)], axis=1)
    visT = np.concatenate([res.results[j]["o_visT"] for j in range(NCORES)], axis=1)
    out_amr = np.ascontiguousarray(amrT.T).reshape(B, NA, D)
    out_vis = np.ascontiguousarray(visT.T).reshape(B, NV, D)
    return out_amr, out_vis
